# revision 14
# baseline (speedup 1.0000x reference)
"""Trainium2 Bass kernel for nn_DeepBKT (4-layer DeepBKT-style transformer).

Sharding: pure data-parallel over batch. B=32 sequences -> 8 NeuronCores x 4
sequences. Weights replicated. No collectives.

v2 design (vs v1 baseline at 1577us):
  - All matmul operands bf16 (stationaries get fast-weight-load, no f32r
    small-N penalty, LDWEIGHTS stream 4x lighter). State x kept in bf16;
    psum accumulation stays f32. Measured numpy rel err ~2e-3 (gate 2e-2).
  - Swapped PV: stationary = eT block [j,i-block], moving = v_ext [j,65]
    -> ctx lands [i, dk] with the softmax denominator as a per-partition
    COLUMN (ones-column trick). Kills the PartitionBroadcast + row-extract
    + wide-reciprocal + wide-multiply denominator pipeline of v1; the
    normalize folds into the psum-evacuation tensor_scalar.
  - forget-rate gate folded into the EXP activation's per-partition scale.
  - FFN weights DMA'd once per layer (v1 re-streamed per sequence: 128MB).
  - Attention(b) emission interleaved with projections of b+1 so the PE
    keeps running through the DVE/ACT-bound softmax stretches.
  - psum->sbuf evacuation copies spread across ACT/DVE/GpSimd by role.
"""

import sys

for _p in ("/opt/trn_rl_repo",):
    if _p not in sys.path:
        sys.path.insert(0, _p)

import numpy as np

import concourse.bacc as bacc
import concourse.bass as bass
import concourse.tile as tile
import concourse.mybir as mybir
from concourse.masks import make_identity

import concourse.tile_utils as tile_utils

tile_utils.max_sbuf_usage = 208 * 1024

F32 = mybir.dt.float32
F32R = mybir.dt.float32r
BF16 = mybir.dt.bfloat16
AF = mybir.ActivationFunctionType
ALU = mybir.AluOpType

P = 128
S, D, H, FF = 512, 512, 8, 2048
DK = D // H  # 64
NT = S // P  # 4 i/j tiles
DT = D // P  # 4 d tiles
NKF = FF // P  # 16 ff tiles
EPS = 1e-5
NEG_BIG = -1e30
N_CORES = 8


def build(L=4, NB=4):
    nc = bacc.Bacc("TRN2", target_bir_lowering=False, debug=False,
                   num_devices=N_CORES)

    q_d = nc.dram_tensor("q", [NB, S, D], F32, kind="ExternalInput")
    qa_d = nc.dram_tensor("qa", [NB, S, D], F32, kind="ExternalInput")
    pid_d = nc.dram_tensor("pid", [NB, S, S], F32, kind="ExternalInput")
    fr_d = nc.dram_tensor("fr", [NB, S], F32, kind="ExternalInput")
    pos_d = nc.dram_tensor("pos", [S, D], F32, kind="ExternalInput")
    wk_d = nc.dram_tensor("Wk", [L, D, D], BF16, kind="ExternalInput")
    wv_d = nc.dram_tensor("Wv", [L, D, D], BF16, kind="ExternalInput")
    wo_d = nc.dram_tensor("Wo", [L, D, D], BF16, kind="ExternalInput")
    w1_d = nc.dram_tensor("W1", [L, D, FF], BF16, kind="ExternalInput")
    w2_d = nc.dram_tensor("W2", [L, FF, D], BF16, kind="ExternalInput")
    out_d = nc.dram_tensor("out", [NB, S, D], F32, kind="ExternalOutput")

    with tile.TileContext(nc) as tc:
        with (
            tc.tile_pool(name="const", bufs=1) as constp,
            tc.tile_pool(name="state", bufs=1) as statep,
            tc.tile_pool(name="res", bufs=1) as resp,
            tc.tile_pool(name="wpool", bufs=1) as wp,
            tc.tile_pool(name="work", bufs=1) as workp,
            tc.tile_pool(name="bigf", bufs=2) as bigp,
            tc.tile_pool(name="small", bufs=6) as smallp,
            tc.tile_pool(name="ps", bufs=8, space="PSUM") as psp,
        ):
            identb = constp.tile([P, P], BF16, tag="identb")
            make_identity(nc, identb)
            eps_t = constp.tile([P, 1], F32, tag="eps")
            nc.vector.memset(eps_t, EPS)

            # ---------------- helpers ----------------
            def transpose4(src_of_it, dst, evac):
                """src_of_it(it) -> AP [128,512] bf16 (seq-major block).
                dst [128, DT, 512] bf16 feature-major. evac: 'act'|'dve'|'gp'"""
                for c in range(DT):
                    ps = psp.tile([P, S], BF16, tag="ps", name="tps")
                    for it in range(NT):
                        nc.tensor.transpose(
                            ps[:, it * P:(it + 1) * P],
                            src_of_it(it)[:, c * P:(c + 1) * P],
                            identb,
                        )
                    if evac == "act" or (evac == "mix" and c % 2 == 0):
                        nc.scalar.copy(out=dst[:, c, :], in_=ps[:])
                    else:
                        nc.vector.tensor_copy(out=dst[:, c, :], in_=ps[:])

            def ln_apply(t, rowsum, dst):
                """LayerNorm over free dim. t [128,512] bf16 pre-LN values,
                rowsum [128,1] f32 = sum over free. Writes normalized dst."""
                mean_neg = smallp.tile([P, 1], F32, tag="mneg")
                nc.scalar.mul(out=mean_neg, in_=rowsum, mul=-1.0 / D)
                var_s = smallp.tile([P, 1], F32, tag="vars")
                sq_scr = workp.tile([P, S], BF16, tag="sp", bufs=6,
                                    name="sqscr")
                nc.scalar.activation(out=sq_scr, in_=t, func=AF.Square,
                                     bias=mean_neg, scale=1.0,
                                     accum_out=var_s)
                std = smallp.tile([P, 1], F32, tag="std")
                nc.scalar.activation(out=std, in_=var_s, func=AF.Sqrt,
                                     bias=eps_t, scale=1.0 / D)
                rstd = smallp.tile([P, 1], F32, tag="rstd")
                nc.vector.reciprocal(out=rstd, in_=std)
                nc.vector.tensor_scalar(out=dst, in0=t, scalar1=mean_neg,
                                        scalar2=rstd, op0=ALU.add,
                                        op1=ALU.mult)

            # ---------------- resident state ----------------
            x_tiles = {}   # b -> [NT] state APs [128,512] bf16 seq-major
            yTs, teTs, frs = {}, {}, {}

            pos_t = bigp.tile([P, NT, D], F32, tag="big", name="post")
            nc.sync.dma_start(
                out=pos_t[:],
                in_=pos_d[:].rearrange("(it p) d -> p it d", p=P))

            for b in range(NB):
                qt = bigp.tile([P, NT, D], F32, tag="big", name="qt")
                nc.sync.dma_start(
                    out=qt[:], in_=q_d[b].rearrange("(it p) d -> p it d", p=P))
                xb = []
                for it in range(NT):
                    xt = statep.tile([P, D], BF16, tag="x", bufs=20, name="xt")
                    nc.vector.tensor_add(out=xt[:], in0=qt[:, it, :],
                                         in1=pos_t[:, it, :])
                    xb.append(xt)
                x_tiles[b] = xb

                yt = bigp.tile([P, NT, D], F32, tag="big", name="yt")
                nc.sync.dma_start(
                    out=yt[:], in_=qa_d[b].rearrange("(it p) d -> p it d", p=P))
                yb = workp.tile([P, NT, D], BF16, tag="eT", bufs=4, name="yb")
                for it in range(NT):
                    nc.vector.tensor_add(out=yb[:, it, :], in0=yt[:, it, :],
                                         in1=pos_t[:, it, :])
                yT = resp.tile([P, DT, S], BF16, tag="yT", bufs=NB, name="yT")
                transpose4(lambda it: yb[:, it, :], yT, "act")
                yTs[b] = yT

                pt = bigp.tile([P, NT, S], F32, tag="big", name="pt")
                nc.sync.dma_start(
                    out=pt[:],
                    in_=pid_d[b].rearrange("(it p) j -> p it j", p=P))
                ptb = workp.tile([P, NT, S], BF16, tag="eT", bufs=4,
                                 name="ptb")
                for it in range(NT):
                    nc.scalar.activation(out=pt[:, it, :], in_=pt[:, it, :],
                                         func=AF.Sigmoid)
                    nc.scalar.activation(out=ptb[:, it, :], in_=pt[:, it, :],
                                         func=AF.Exp)
                teT = resp.tile([P, NT, S], BF16, tag="teT", bufs=NB,
                                name="teT")
                transpose4(lambda it: ptb[:, it, :], teT, "dve")
                teTs[b] = teT

                ft = resp.tile([P, NT], F32, tag="frs", bufs=NB, name="ft")
                nc.sync.dma_start(
                    out=ft[:], in_=fr_d[b].rearrange("(t p) -> p t", p=P))
                nc.scalar.mul(out=ft[:], in_=ft[:], mul=1.0 / np.sqrt(DK))
                frs[b] = ft

            # ---------------- per-layer weights ----------------
            wk_t, wv_t, wo_t, w1_t, w2_t = {}, {}, {}, {}, {}

            def load_layer_weights(l):
                wk = wp.tile([P, DT, D], BF16, tag="w3", bufs=6, name="wk")
                nc.sync.dma_start(
                    out=wk[:], in_=wk_d[l].rearrange("(c p) m -> p c m", p=P))
                wv = wp.tile([P, DT, D], BF16, tag="w3", bufs=6, name="wv")
                nc.sync.dma_start(
                    out=wv[:], in_=wv_d[l].rearrange("(c p) m -> p c m", p=P))
                wo = wp.tile([P, DT, D], BF16, tag="w3", bufs=6, name="wo")
                nc.sync.dma_start(
                    out=wo[:], in_=wo_d[l].rearrange("(c p) m -> p c m", p=P))
                w1 = wp.tile([P, DT, FF], BF16, tag="w1", bufs=1, name="w1")
                nc.sync.dma_start(
                    out=w1[:], in_=w1_d[l].rearrange("(c p) f -> p c f", p=P))
                w2 = wp.tile([P, NKF, D], BF16, tag="w2", bufs=1, name="w2")
                nc.sync.dma_start(
                    out=w2[:], in_=w2_d[l].rearrange("(c p) d -> p c d", p=P))
                wk_t[l], wv_t[l], wo_t[l] = wk, wv, wo
                w1_t[l], w2_t[l] = w1, w2

            # ---------------- projection chunks (qkT, vext for (l,b)) ------
            proj_out = {}  # (l,b) -> (qkT, vext)

            def make_proj_chunks(l, b):
                """Returns list of closures; running all of them computes
                qkT[d,i] and vext[j,(h,dk+1)] for (l, b)."""
                xb = x_tiles[b]
                xT = workp.tile([P, DT, S], BF16, tag="xT", bufs=2, name="xT")
                qkT = workp.tile([P, DT, S], BF16, tag="qkT", bufs=2,
                                 name="qkT")
                vext = workp.tile([P, NT, H, DK + 1], BF16, tag="vext",
                                  bufs=2, name="vext")
                proj_out[(l, b)] = (qkT, vext)
                chunks = []

                def xt_chunk(c):
                    def run():
                        ps = psp.tile([P, S], BF16, tag="ps", name="xtps")
                        for it in range(NT):
                            nc.tensor.transpose(
                                ps[:, it * P:(it + 1) * P],
                                xb[it][:, c * P:(c + 1) * P], identb)
                        nc.vector.tensor_copy(out=xT[:, c, :], in_=ps[:])
                    return run

                def qk_chunk(mt):
                    def run():
                        ps = psp.tile([P, S], F32, tag="ps", name="qkps")
                        for c in range(DT):
                            nc.tensor.matmul(
                                ps[:], wk_t[l][:, c, mt * P:(mt + 1) * P],
                                xT[:, c, :], start=(c == 0),
                                stop=(c == DT - 1))
                        nc.scalar.copy(out=qkT[:, mt, :], in_=ps[:])
                    return run

                def v_chunk(it):
                    def run():
                        if it == 0:
                            nc.vector.memset(vext[:, :, :, DK:DK + 1], 1.0)
                        ps = psp.tile([P, S], F32, tag="ps", name="vps")
                        for c in range(DT):
                            nc.tensor.matmul(
                                ps[:], yTs[b][:, c, it * P:(it + 1) * P],
                                wv_t[l][:, c, :], start=(c == 0),
                                stop=(c == DT - 1))
                        nc.vector.tensor_copy(
                            out=vext[:, it, :, 0:DK],
                            in_=ps[:].rearrange("p (h k) -> p h k", h=H))
                    return run

                for c in range(DT):
                    chunks.append(xt_chunk(c))
                for mt in range(DT):
                    chunks.append(qk_chunk(mt))
                for it in range(NT):
                    chunks.append(v_chunk(it))
                return chunks

            # ---------------- attention for (l, b) ----------------
            def emit_scores(l, b, h):
                """-> eT tile [128, NT, 512] bf16 (j-major tiles)."""
                qkT, _ = proj_out[(l, b)]
                hp0 = (h % 2) * DK
                qh = qkT[hp0:hp0 + DK, h // 2, :]
                eT = workp.tile([P, NT, S], BF16, tag="eT", bufs=4, name="eT")
                for tj in range(NT):
                    i0 = tj * P
                    ni = S - i0
                    sc_ps = psp.tile([P, S], F32, tag="ps", name="scps")
                    nc.tensor.matmul(
                        sc_ps[:, 0:ni], qh[:, i0:i0 + P], qh[:, i0:S],
                        start=True, stop=True)
                    sp = workp.tile([P, S], BF16, tag="sp", bufs=6, name="sp")
                    nc.vector.tensor_mul(
                        out=sp[:, 0:ni], in0=sc_ps[:, 0:ni],
                        in1=teTs[b][:, tj, i0:S])
                    # strict causal mask on the diagonal block: keep j < i
                    nc.gpsimd.affine_select(
                        out=sp[:, 0:P], in_=sp[:, 0:P],
                        compare_op=ALU.is_gt, fill=NEG_BIG,
                        base=0, channel_multiplier=-1,
                        pattern=[[1, P]])
                    nc.scalar.activation(
                        out=eT[:, tj, 0:ni], in_=sp[:, 0:ni], func=AF.Exp,
                        scale=frs[b][:, tj:tj + 1])
                return eT

            def emit_pv(l, b, h, eT, ctxIH):
                """Swapped PV: ctx[i, dk] per i-tile with denominator column.
                Writes normalized ctx into ctxIH[ti][:, h*64:(h+1)*64]."""
                _, vext = proj_out[(l, b)]
                for ti in range(NT):
                    ctx_ps = psp.tile([P, DK + 1], F32, tag="ps", name="ctxps")
                    for tj in range(ti + 1):
                        nc.tensor.matmul(
                            ctx_ps[:],
                            eT[:, tj, (ti - tj) * P:(ti - tj) * P + P],
                            vext[:, tj, h, :],
                            start=(tj == 0), stop=(tj == ti))
                    dinv = smallp.tile([P, 1], F32, tag="dinv", name="dinv")
                    if ti == 0:
                        # only global row i=0 has den==0 (fully masked);
                        # eps keeps 0 * (1/den) = 0 instead of NaN there
                        den = smallp.tile([P, 1], F32, tag="den", name="den")
                        nc.vector.tensor_scalar_add(out=den,
                                                    in0=ctx_ps[:, DK:DK + 1],
                                                    scalar1=1e-37)
                        nc.vector.reciprocal_approx_fast(out=dinv, in_=den)
                    else:
                        nc.vector.reciprocal_approx_fast(
                            out=dinv, in_=ctx_ps[:, DK:DK + 1])
                    nc.vector.tensor_scalar_mul(
                        out=ctxIH[ti][:, h * DK:(h + 1) * DK],
                        in0=ctx_ps[:, 0:DK], scalar1=dinv)

            def attn_units(l, b):
                """Unit closures for attention(l,b), to be woven with the
                previous step's FFN units. ctx transposes lag their PV pair
                so the DVE evacuations have drained by the time they run."""
                ctxIH = [workp.tile([P, D], BF16, tag="ctxIH", bufs=5,
                                    name="ctxIH") for _ in range(NT)]
                ctxT = workp.tile([P, DT, S], BF16, tag="ctxT", bufs=2,
                                  name="ctxT")
                eTs = {}

                def ctx_transpose(c):
                    ps = psp.tile([P, S], BF16, tag="ps", name="ctps")
                    for ti in range(NT):
                        nc.tensor.transpose(
                            ps[:, ti * P:(ti + 1) * P],
                            ctxIH[ti][:, c * P:(c + 1) * P], identb)
                    if c % 2 == 0:
                        nc.scalar.copy(out=ctxT[:, c, :], in_=ps[:])
                    else:
                        nc.vector.tensor_copy(out=ctxT[:, c, :], in_=ps[:])

                def u_scores(h, pv=None, tc=None):
                    # PV first: its psum evacuations drain on DVE while the
                    # PE streams the next head's score matmuls; the lagged
                    # ctx transpose then finds its inputs ready.
                    def run():
                        if pv is not None:
                            emit_pv(l, b, pv, eTs.pop(pv), ctxIH)
                        eTs[h] = emit_scores(l, b, h)
                        if tc is not None:
                            ctx_transpose(tc)
                    return run

                def u_pv(pv, tc=None):
                    def run():
                        emit_pv(l, b, pv, eTs.pop(pv), ctxIH)
                        if tc is not None:
                            ctx_transpose(tc)
                    return run

                def u_outproj(it, tc=None):
                    def run():
                        if tc is not None:
                            ctx_transpose(tc)
                        ps = psp.tile([P, S], F32, tag="ps", name="oprps")
                        for c in range(DT):
                            nc.tensor.matmul(
                                ps[:], ctxT[:, c, it * P:(it + 1) * P],
                                wo_t[l][:, c, :], start=(c == 0),
                                stop=(c == DT - 1))
                        t = workp.tile([P, S], BF16, tag="t", bufs=6,
                                       name="t1")
                        rs = smallp.tile([P, 1], F32, tag="rs")
                        nc.vector.scalar_tensor_tensor(
                            out=t[:], in0=ps[:], scalar=1.0,
                            in1=x_tiles[b][it][:],
                            op0=ALU.mult, op1=ALU.add, accum_out=rs)
                        x1 = statep.tile([P, D], BF16, tag="x", bufs=20,
                                         name="x1")
                        ln_apply(t[:], rs[:], x1[:])
                        x_tiles[b][it] = x1
                    return run

                return [
                    u_scores(0), u_scores(1), u_scores(2, pv=0),
                    u_scores(3, pv=1), u_scores(4, pv=2, tc=0),
                    u_scores(5, pv=3), u_scores(6, pv=4, tc=1),
                    u_scores(7, pv=5),
                    u_pv(6, tc=2), u_pv(7),
                    u_outproj(0, tc=3), u_outproj(1), u_outproj(2),
                    u_outproj(3),
                ]

            # ---------------- FFN units for (l, b) ----------------
            def ffn_units(l, b, last):
                x1b = list(x_tiles[b])
                w1, w2 = w1_t[l], w2_t[l]
                x1T = workp.tile([P, DT, S], BF16, tag="x1T", bufs=2,
                                 name="x1T")
                y2_holder = {}
                units = []

                def u_x1t(c0, c1):
                    def run():
                        for c in (c0, c1):
                            tp = psp.tile([P, S], BF16, tag="ps",
                                          name="x1tps")
                            for it in range(NT):
                                nc.tensor.transpose(
                                    tp[:, it * P:(it + 1) * P],
                                    x1b[it][:, c * P:(c + 1) * P], identb)
                            if c % 2 == 0:
                                nc.scalar.copy(out=x1T[:, c, :], in_=tp[:])
                            else:
                                nc.vector.tensor_copy(out=x1T[:, c, :],
                                                      in_=tp[:])
                        if c0 == 0:
                            y2_holder["ps"] = [
                                psp.tile([P, S], F32, tag="ps", name="y2ps")
                                for _ in range(NT)]
                    return run

                hTs = {}

                def u_ffn1(kf):
                    def run():
                        h_ps = psp.tile([P, S], F32, tag="ps", name="hps")
                        for c in range(DT):
                            nc.tensor.matmul(
                                h_ps[:], w1[:, c, kf * P:(kf + 1) * P],
                                x1T[:, c, :], start=(c == 0),
                                stop=(c == DT - 1))
                        hT = workp.tile([P, S], BF16, tag="hT", bufs=3,
                                        name="hT")
                        if kf % 2 == 0:
                            nc.scalar.activation(out=hT[:], in_=h_ps[:],
                                                 func=AF.Relu)
                        else:
                            nc.vector.tensor_scalar_max(
                                out=hT[:], in0=h_ps[:], scalar1=0.0)
                        hTs[kf] = hT
                    return run

                def u_ffn2(kf):
                    def run():
                        phT = hTs.pop(kf)
                        for it in range(NT):
                            nc.tensor.matmul(
                                y2_holder["ps"][it][:],
                                phT[:, it * P:(it + 1) * P],
                                w2[:, kf, :], start=(kf == 0),
                                stop=(kf == NKF - 1))
                    return run

                def u_ln2(it):
                    def run():
                        t2 = workp.tile([P, S], BF16, tag="t", bufs=6,
                                        name="t2")
                        rs2 = smallp.tile([P, 1], F32, tag="rs")
                        nc.vector.scalar_tensor_tensor(
                            out=t2[:], in0=y2_holder["ps"][it][:], scalar=1.0,
                            in1=x1b[it][:], op0=ALU.mult, op1=ALU.add,
                            accum_out=rs2)
                        if last:
                            x2 = statep.tile([P, D], F32, tag="xout", bufs=3,
                                             name="x2o")
                            ln_apply(t2[:], rs2[:], x2[:])
                            nc.sync.dma_start(
                                out=out_d[b, it * P:(it + 1) * P, :],
                                in_=x2[:])
                        else:
                            x2 = statep.tile([P, D], BF16, tag="x", bufs=20,
                                             name="x2")
                            ln_apply(t2[:], rs2[:], x2[:])
                        x_tiles[b][it] = x2
                    return run

                units.append(u_x1t(0, 1))
                units.append(u_x1t(2, 3))
                # ffn1(kf) ... ffn2 lags by 2 for relu latency
                for kf in range(NKF):
                    units.append(u_ffn1(kf))
                    if kf >= 2:
                        units.append(u_ffn2(kf - 2))
                units.append(u_ffn2(NKF - 2))
                units.append(u_ffn2(NKF - 1))
                for it in range(NT):
                    units.append(u_ln2(it))
                return units

            def weave(lead, *streams):
                """Emit unit streams proportionally interleaved, preserving
                within-stream order (deps only point backward). The first
                `lead` units of stream 0 are emitted up front so the new
                attention stretch gets ahead of the lagging FFN stream."""
                streams = [list(s) for s in streams if s]
                if not streams:
                    return
                idx = [0] * len(streams)
                for _ in range(min(lead, len(streams[0]))):
                    streams[0][idx[0]]()
                    idx[0] += 1
                total = sum(len(s) - i for s, i in zip(streams, idx))
                for _ in range(total):
                    best, bestv = 0, -1.0
                    for k, s in enumerate(streams):
                        if idx[k] < len(s):
                            v = (len(s) - idx[k]) / len(s)
                            if v > bestv:
                                best, bestv = k, v
                    streams[best][idx[best]]()
                    idx[best] += 1

            # ---------------- main schedule ----------------
            load_layer_weights(0)
            load_layer_weights(1)
            for ch in make_proj_chunks(0, 0):
                ch()
            prev_ffn = None
            for l in range(L):
                if 1 <= l and l + 1 < L:
                    load_layer_weights(l + 1)  # prefetch, overlaps compute
                for b in range(NB):
                    if b < NB - 1:
                        nxt = make_proj_chunks(l, b + 1)
                    elif l < L - 1:
                        nxt = make_proj_chunks(l + 1, 0)
                    else:
                        nxt = []
                    au = attn_units(l, b)
                    weave(2, au, prev_ffn, nxt)
                    prev_ffn = ffn_units(l, b, last=(l == L - 1))
            for u in prev_ffn:
                u()

    nc.compile()
    return nc


_BUILD_CACHE = {}


def _get_nc(L, NB):
    key = (L, NB)
    if key not in _BUILD_CACHE:
        _BUILD_CACHE[key] = build(L, NB)
    return _BUILD_CACHE[key]


def make_in_maps(inputs, L=4, NB=4, n_cores=N_CORES):
    """Shard full inputs into per-core in_maps."""
    import ml_dtypes
    f32 = np.float32
    bf = ml_dtypes.bfloat16
    q = np.ascontiguousarray(np.asarray(inputs["q_embed_data"], f32))
    qa = np.ascontiguousarray(np.asarray(inputs["qa_embed_data"], f32))
    pid = np.ascontiguousarray(np.asarray(inputs["pid_embed_data"], f32))
    fr = np.asarray(inputs["forget_rate"], f32)[:, :, 0]
    # guard: exact-zero forget rate would break the mask-fill folded into
    # the EXP scale (exp(0 * -1e30) = 1); reference gives uniform attention
    # over the past for fr == 0, which fr = 1e-20 reproduces.
    fr = np.ascontiguousarray(np.maximum(fr, 1e-20))
    pos = np.ascontiguousarray(np.asarray(inputs["pos_emb"], f32)[0])
    wdict = {}
    for n in ["Wk", "Wv", "Wo", "W1", "W2"]:
        wdict[n] = np.ascontiguousarray(
            np.asarray(inputs[n], f32).astype(bf))

    # biases / LN affine are zero/one in this model; verify and fall back
    # is not implemented (asserted host-side).
    for n in ["bk", "bv", "bo", "b1", "b2", "ln1_b", "ln2_b"]:
        assert np.all(np.asarray(inputs[n]) == 0.0), f"nonzero {n}"
    for n in ["ln1_g", "ln2_g"]:
        assert np.all(np.asarray(inputs[n]) == 1.0), f"non-unit {n}"

    in_maps = []
    for c in range(n_cores):
        sl = slice(c * NB, (c + 1) * NB)
        m = {
            "q": q[sl], "qa": qa[sl], "pid": pid[sl], "fr": fr[sl],
            "pos": pos,
            "Wk": wdict["Wk"][:L], "Wv": wdict["Wv"][:L],
            "Wo": wdict["Wo"][:L],
            "W1": wdict["W1"][:L], "W2": wdict["W2"][:L],
        }
        in_maps.append(m)
    return in_maps


def kernel(**inputs):
    from concourse.bass_utils import run_bass_kernel_spmd

    B = int(np.asarray(inputs["q_embed_data"]).shape[0])
    NB = B // N_CORES
    L = int(np.asarray(inputs["Wk"]).shape[0])
    in_maps = make_in_maps(inputs, L=L, NB=NB)
    nc = _get_nc(L, NB)
    res = run_bass_kernel_spmd(nc, in_maps, core_ids=list(range(N_CORES)))
    out = np.concatenate([res.results[c]["out"] for c in range(N_CORES)],
                         axis=0)
    return out.astype(np.float32)


# revision 16
# speedup vs baseline: 1.0563x; 1.0563x over previous
"""Trainium2 Bass kernel for nn_DeepBKT (4-layer DeepBKT-style transformer).

Sharding: pure data-parallel over batch. B=32 sequences -> 8 NeuronCores x 4
sequences. Weights replicated. No collectives.

v2 design (vs v1 baseline at 1577us):
  - All matmul operands bf16 (stationaries get fast-weight-load, no f32r
    small-N penalty, LDWEIGHTS stream 4x lighter). State x kept in bf16;
    psum accumulation stays f32. Measured numpy rel err ~2e-3 (gate 2e-2).
  - Swapped PV: stationary = eT block [j,i-block], moving = v_ext [j,65]
    -> ctx lands [i, dk] with the softmax denominator as a per-partition
    COLUMN (ones-column trick). Kills the PartitionBroadcast + row-extract
    + wide-reciprocal + wide-multiply denominator pipeline of v1; the
    normalize folds into the psum-evacuation tensor_scalar.
  - forget-rate gate folded into the EXP activation's per-partition scale.
  - FFN weights DMA'd once per layer (v1 re-streamed per sequence: 128MB).
  - Attention(b) emission interleaved with projections of b+1 so the PE
    keeps running through the DVE/ACT-bound softmax stretches.
  - psum->sbuf evacuation copies spread across ACT/DVE/GpSimd by role.
"""

import sys

for _p in ("/opt/trn_rl_repo",):
    if _p not in sys.path:
        sys.path.insert(0, _p)

import numpy as np

import concourse.bacc as bacc
import concourse.bass as bass
import concourse.tile as tile
import concourse.mybir as mybir
from concourse.masks import make_identity

import concourse.tile_utils as tile_utils

tile_utils.max_sbuf_usage = 208 * 1024

F32 = mybir.dt.float32
F32R = mybir.dt.float32r
BF16 = mybir.dt.bfloat16
AF = mybir.ActivationFunctionType
ALU = mybir.AluOpType

P = 128
S, D, H, FF = 512, 512, 8, 2048
DK = D // H  # 64
NT = S // P  # 4 i/j tiles
DT = D // P  # 4 d tiles
NKF = FF // P  # 16 ff tiles
EPS = 1e-5
NEG_BIG = -1e30
N_CORES = 8


def build(L=4, NB=4):
    nc = bacc.Bacc("TRN2", target_bir_lowering=False, debug=False,
                   num_devices=N_CORES)

    q_d = nc.dram_tensor("q", [NB, S, D], F32, kind="ExternalInput")
    qa_d = nc.dram_tensor("qa", [NB, S, D], F32, kind="ExternalInput")
    pid_d = nc.dram_tensor("pid", [NB, S, S], F32, kind="ExternalInput")
    fr_d = nc.dram_tensor("fr", [NB, S], F32, kind="ExternalInput")
    pos_d = nc.dram_tensor("pos", [S, D], F32, kind="ExternalInput")
    wk_d = nc.dram_tensor("Wk", [L, D, D], BF16, kind="ExternalInput")
    wv_d = nc.dram_tensor("Wv", [L, D, D], BF16, kind="ExternalInput")
    wo_d = nc.dram_tensor("Wo", [L, D, D], BF16, kind="ExternalInput")
    w1_d = nc.dram_tensor("W1", [L, D, FF], BF16, kind="ExternalInput")
    w2_d = nc.dram_tensor("W2", [L, FF, D], BF16, kind="ExternalInput")
    out_d = nc.dram_tensor("out", [NB, S, D], F32, kind="ExternalOutput")

    with tile.TileContext(nc) as tc:
        with (
            tc.tile_pool(name="const", bufs=1) as constp,
            tc.tile_pool(name="state", bufs=1) as statep,
            tc.tile_pool(name="res", bufs=1) as resp,
            tc.tile_pool(name="wpool", bufs=1) as wp,
            tc.tile_pool(name="work", bufs=1) as workp,
            tc.tile_pool(name="bigf", bufs=2) as bigp,
            tc.tile_pool(name="small", bufs=6) as smallp,
            tc.tile_pool(name="ps", bufs=8, space="PSUM") as psp,
        ):
            identb = constp.tile([P, P], BF16, tag="identb")
            make_identity(nc, identb)
            eps_t = constp.tile([P, 1], F32, tag="eps")
            nc.vector.memset(eps_t, EPS)

            # ---------------- helpers ----------------
            def transpose4(src_of_it, dst, evac):
                """src_of_it(it) -> AP [128,512] bf16 (seq-major block).
                dst [128, DT, 512] bf16 feature-major. evac: 'act'|'dve'|'gp'"""
                for c in range(DT):
                    ps = psp.tile([P, S], BF16, tag="ps", name="tps")
                    for it in range(NT):
                        nc.tensor.transpose(
                            ps[:, it * P:(it + 1) * P],
                            src_of_it(it)[:, c * P:(c + 1) * P],
                            identb,
                        )
                    if evac == "act" or (evac == "mix" and c % 2 == 0):
                        nc.scalar.copy(out=dst[:, c, :], in_=ps[:])
                    else:
                        nc.vector.tensor_copy(out=dst[:, c, :], in_=ps[:])

            def ln_apply(t, rowsum, dst):
                """LayerNorm over free dim. t [128,512] bf16 pre-LN values,
                rowsum [128,1] f32 = sum over free. Writes normalized dst."""
                mean_neg = smallp.tile([P, 1], F32, tag="mneg")
                nc.scalar.mul(out=mean_neg, in_=rowsum, mul=-1.0 / D)
                var_s = smallp.tile([P, 1], F32, tag="vars")
                sq_scr = workp.tile([P, S], BF16, tag="sp", bufs=6,
                                    name="sqscr")
                nc.scalar.activation(out=sq_scr, in_=t, func=AF.Square,
                                     bias=mean_neg, scale=1.0,
                                     accum_out=var_s)
                std = smallp.tile([P, 1], F32, tag="std")
                nc.scalar.activation(out=std, in_=var_s, func=AF.Sqrt,
                                     bias=eps_t, scale=1.0 / D)
                rstd = smallp.tile([P, 1], F32, tag="rstd")
                nc.vector.reciprocal(out=rstd, in_=std)
                nc.vector.tensor_scalar(out=dst, in0=t, scalar1=mean_neg,
                                        scalar2=rstd, op0=ALU.add,
                                        op1=ALU.mult)

            # ---------------- resident state ----------------
            x_tiles = {}   # b -> [NT] state APs [128,512] bf16 seq-major
            yTs, teTs, frs = {}, {}, {}

            pos_t = bigp.tile([P, NT, D], F32, tag="big", name="post")
            nc.sync.dma_start(
                out=pos_t[:],
                in_=pos_d[:].rearrange("(it p) d -> p it d", p=P))

            for b in range(NB):
                qt = bigp.tile([P, NT, D], F32, tag="big", name="qt")
                nc.sync.dma_start(
                    out=qt[:], in_=q_d[b].rearrange("(it p) d -> p it d", p=P))
                xb = []
                for it in range(NT):
                    xt = statep.tile([P, D], BF16, tag="x", bufs=20, name="xt")
                    nc.vector.tensor_add(out=xt[:], in0=qt[:, it, :],
                                         in1=pos_t[:, it, :])
                    xb.append(xt)
                x_tiles[b] = xb

                yt = bigp.tile([P, NT, D], F32, tag="big", name="yt")
                nc.sync.dma_start(
                    out=yt[:], in_=qa_d[b].rearrange("(it p) d -> p it d", p=P))
                yb = workp.tile([P, NT, D], BF16, tag="eT", bufs=4, name="yb")
                for it in range(NT):
                    nc.vector.tensor_add(out=yb[:, it, :], in0=yt[:, it, :],
                                         in1=pos_t[:, it, :])
                yT = resp.tile([P, DT, S], BF16, tag="yT", bufs=NB, name="yT")
                transpose4(lambda it: yb[:, it, :], yT, "act")
                yTs[b] = yT

                pt = bigp.tile([P, NT, S], F32, tag="big", name="pt")
                nc.sync.dma_start(
                    out=pt[:],
                    in_=pid_d[b].rearrange("(it p) j -> p it j", p=P))
                ptb = workp.tile([P, NT, S], BF16, tag="eT", bufs=4,
                                 name="ptb")
                for it in range(NT):
                    nc.scalar.activation(out=pt[:, it, :], in_=pt[:, it, :],
                                         func=AF.Sigmoid)
                    nc.scalar.activation(out=ptb[:, it, :], in_=pt[:, it, :],
                                         func=AF.Exp)
                teT = resp.tile([P, NT, S], BF16, tag="teT", bufs=NB,
                                name="teT")
                transpose4(lambda it: ptb[:, it, :], teT, "dve")
                teTs[b] = teT

                ft = resp.tile([P, NT], F32, tag="frs", bufs=NB, name="ft")
                nc.sync.dma_start(
                    out=ft[:], in_=fr_d[b].rearrange("(t p) -> p t", p=P))
                nc.scalar.mul(out=ft[:], in_=ft[:], mul=1.0 / np.sqrt(DK))
                frs[b] = ft

            # ---------------- per-layer weights ----------------
            wk_t, wv_t, wo_t, w1_t, w2_t = {}, {}, {}, {}, {}

            def load_layer_weights(l):
                wk = wp.tile([P, DT, D], BF16, tag="w3", bufs=6, name="wk")
                nc.sync.dma_start(
                    out=wk[:], in_=wk_d[l].rearrange("(c p) m -> p c m", p=P))
                wv = wp.tile([P, DT, D], BF16, tag="w3", bufs=6, name="wv")
                nc.sync.dma_start(
                    out=wv[:], in_=wv_d[l].rearrange("(c p) m -> p c m", p=P))
                wo = wp.tile([P, DT, D], BF16, tag="w3", bufs=6, name="wo")
                nc.sync.dma_start(
                    out=wo[:], in_=wo_d[l].rearrange("(c p) m -> p c m", p=P))
                w1 = wp.tile([P, DT, FF], BF16, tag="w1", bufs=1, name="w1")
                nc.sync.dma_start(
                    out=w1[:], in_=w1_d[l].rearrange("(c p) f -> p c f", p=P))
                w2 = wp.tile([P, NKF, D], BF16, tag="w2", bufs=1, name="w2")
                nc.sync.dma_start(
                    out=w2[:], in_=w2_d[l].rearrange("(c p) d -> p c d", p=P))
                wk_t[l], wv_t[l], wo_t[l] = wk, wv, wo
                w1_t[l], w2_t[l] = w1, w2

            # ---------------- projection chunks (qkT, vext for (l,b)) ------
            proj_out = {}  # (l,b) -> (qkT, vext)

            def make_proj_chunks(l, b):
                """Returns list of closures; running all of them computes
                qkT[d,i] and vext[j,(h,dk+1)] for (l, b)."""
                xb = x_tiles[b]
                xT = workp.tile([P, DT, S], BF16, tag="xT", bufs=2, name="xT")
                qkT = workp.tile([P, DT, S], BF16, tag="qkT", bufs=2,
                                 name="qkT")
                vext = workp.tile([P, NT, H, DK + 1], BF16, tag="vext",
                                  bufs=2, name="vext")
                proj_out[(l, b)] = (qkT, vext)
                chunks = []

                def xt_chunk(c):
                    def run():
                        ps = psp.tile([P, S], BF16, tag="ps", name="xtps")
                        for it in range(NT):
                            nc.tensor.transpose(
                                ps[:, it * P:(it + 1) * P],
                                xb[it][:, c * P:(c + 1) * P], identb)
                        nc.vector.tensor_copy(out=xT[:, c, :], in_=ps[:])
                    return run

                def qk_chunk(mt):
                    def run():
                        ps = psp.tile([P, S], F32, tag="ps", name="qkps")
                        for c in range(DT):
                            nc.tensor.matmul(
                                ps[:], wk_t[l][:, c, mt * P:(mt + 1) * P],
                                xT[:, c, :], start=(c == 0),
                                stop=(c == DT - 1))
                        nc.scalar.copy(out=qkT[:, mt, :], in_=ps[:])
                    return run

                def v_chunk(it):
                    def run():
                        if it == 0:
                            nc.vector.memset(vext[:, :, :, DK:DK + 1], 1.0)
                        ps = psp.tile([P, S], F32, tag="ps", name="vps")
                        for c in range(DT):
                            nc.tensor.matmul(
                                ps[:], yTs[b][:, c, it * P:(it + 1) * P],
                                wv_t[l][:, c, :], start=(c == 0),
                                stop=(c == DT - 1))
                        nc.vector.tensor_copy(
                            out=vext[:, it, :, 0:DK],
                            in_=ps[:].rearrange("p (h k) -> p h k", h=H))
                    return run

                for c in range(DT):
                    chunks.append(xt_chunk(c))
                for mt in range(DT):
                    chunks.append(qk_chunk(mt))
                for it in range(NT):
                    chunks.append(v_chunk(it))
                return chunks

            # ---------------- attention for (l, b) ----------------
            def emit_scores(l, b, h):
                """-> eT tile [128, NT, 512] bf16 (j-major tiles)."""
                qkT, _ = proj_out[(l, b)]
                hp0 = (h % 2) * DK
                qh = qkT[hp0:hp0 + DK, h // 2, :]
                eT = workp.tile([P, NT, S], BF16, tag="eT", bufs=4, name="eT")
                for tj in range(NT):
                    i0 = tj * P
                    ni = S - i0
                    sc_ps = psp.tile([P, S], F32, tag="ps", name="scps")
                    nc.tensor.matmul(
                        sc_ps[:, 0:ni], qh[:, i0:i0 + P], qh[:, i0:S],
                        start=True, stop=True)
                    sp = workp.tile([P, S], BF16, tag="sp", bufs=6, name="sp")
                    nc.vector.tensor_mul(
                        out=sp[:, 0:ni], in0=sc_ps[:, 0:ni],
                        in1=teTs[b][:, tj, i0:S])
                    # strict causal mask on the diagonal block: keep j < i
                    nc.gpsimd.affine_select(
                        out=sp[:, 0:P], in_=sp[:, 0:P],
                        compare_op=ALU.is_gt, fill=NEG_BIG,
                        base=0, channel_multiplier=-1,
                        pattern=[[1, P]])
                    nc.scalar.activation(
                        out=eT[:, tj, 0:ni], in_=sp[:, 0:ni], func=AF.Exp,
                        scale=frs[b][:, tj:tj + 1])
                return eT

            def emit_pv(l, b, h, eT, ctxIH):
                """Swapped PV: ctx[i, dk] per i-tile with denominator column.
                Writes normalized ctx into ctxIH[ti][:, h*64:(h+1)*64]."""
                _, vext = proj_out[(l, b)]
                for ti in range(NT):
                    ctx_ps = psp.tile([P, DK + 1], F32, tag="ps", name="ctxps")
                    for tj in range(ti + 1):
                        nc.tensor.matmul(
                            ctx_ps[:],
                            eT[:, tj, (ti - tj) * P:(ti - tj) * P + P],
                            vext[:, tj, h, :],
                            start=(tj == 0), stop=(tj == ti))
                    dinv = smallp.tile([P, 1], F32, tag="dinv", name="dinv")
                    if ti == 0:
                        # only global row i=0 has den==0 (fully masked);
                        # eps keeps 0 * (1/den) = 0 instead of NaN there
                        den = smallp.tile([P, 1], F32, tag="den", name="den")
                        nc.vector.tensor_scalar_add(out=den,
                                                    in0=ctx_ps[:, DK:DK + 1],
                                                    scalar1=1e-37)
                        nc.vector.reciprocal_approx_fast(out=dinv, in_=den)
                    else:
                        nc.vector.reciprocal_approx_fast(
                            out=dinv, in_=ctx_ps[:, DK:DK + 1])
                    nc.vector.tensor_scalar_mul(
                        out=ctxIH[ti][:, h * DK:(h + 1) * DK],
                        in0=ctx_ps[:, 0:DK], scalar1=dinv)

            def attn_units(l, b):
                """Unit closures for attention(l,b), to be woven with the
                previous step's FFN units. ctx transposes lag their PV pair
                so the DVE evacuations have drained by the time they run."""
                ctxIH = [workp.tile([P, D], BF16, tag="ctxIH", bufs=5,
                                    name="ctxIH") for _ in range(NT)]
                ctxT = workp.tile([P, DT, S], BF16, tag="ctxT", bufs=2,
                                  name="ctxT")
                eTs = {}

                def ctx_transpose(c):
                    ps = psp.tile([P, S], BF16, tag="ps", name="ctps")
                    for ti in range(NT):
                        nc.tensor.transpose(
                            ps[:, ti * P:(ti + 1) * P],
                            ctxIH[ti][:, c * P:(c + 1) * P], identb)
                    if c % 2 == 0:
                        nc.scalar.copy(out=ctxT[:, c, :], in_=ps[:])
                    else:
                        nc.vector.tensor_copy(out=ctxT[:, c, :], in_=ps[:])

                def u_scores(h, pv=None, tc=None):
                    # PV first: its psum evacuations drain on DVE while the
                    # PE streams the next head's score matmuls; the lagged
                    # ctx transpose then finds its inputs ready.
                    def run():
                        if pv is not None:
                            emit_pv(l, b, pv, eTs.pop(pv), ctxIH)
                        eTs[h] = emit_scores(l, b, h)
                        if tc is not None:
                            ctx_transpose(tc)
                    return run

                def u_pv(pv, tc=None):
                    def run():
                        emit_pv(l, b, pv, eTs.pop(pv), ctxIH)
                        if tc is not None:
                            ctx_transpose(tc)
                    return run

                def u_outproj(it, tc=None):
                    def run():
                        if tc is not None:
                            ctx_transpose(tc)
                        ps = psp.tile([P, S], F32, tag="ps", name="oprps")
                        for c in range(DT):
                            nc.tensor.matmul(
                                ps[:], ctxT[:, c, it * P:(it + 1) * P],
                                wo_t[l][:, c, :], start=(c == 0),
                                stop=(c == DT - 1))
                        t = workp.tile([P, S], BF16, tag="t", bufs=6,
                                       name="t1")
                        rs = smallp.tile([P, 1], F32, tag="rs")
                        nc.vector.scalar_tensor_tensor(
                            out=t[:], in0=ps[:], scalar=1.0,
                            in1=x_tiles[b][it][:],
                            op0=ALU.mult, op1=ALU.add, accum_out=rs)
                        x1 = statep.tile([P, D], BF16, tag="x", bufs=20,
                                         name="x1")
                        ln_apply(t[:], rs[:], x1[:])
                        x_tiles[b][it] = x1
                    return run

                return [
                    u_scores(0), u_scores(1), u_scores(2, pv=0),
                    u_scores(3, pv=1), u_scores(4, pv=2),
                    u_scores(5, pv=3, tc=0), u_scores(6, pv=4),
                    u_scores(7, pv=5, tc=1),
                    u_pv(6, tc=2), u_pv(7),
                    u_outproj(0, tc=3), u_outproj(1), u_outproj(2),
                    u_outproj(3),
                ]

            # ---------------- FFN units for (l, b) ----------------
            def ffn_units(l, b, last):
                x1b = list(x_tiles[b])
                w1, w2 = w1_t[l], w2_t[l]
                x1T = workp.tile([P, DT, S], BF16, tag="x1T", bufs=2,
                                 name="x1T")
                y2_holder = {}
                units = []

                def u_x1t(c0, c1):
                    def run():
                        for c in (c0, c1):
                            tp = psp.tile([P, S], BF16, tag="ps",
                                          name="x1tps")
                            for it in range(NT):
                                nc.tensor.transpose(
                                    tp[:, it * P:(it + 1) * P],
                                    x1b[it][:, c * P:(c + 1) * P], identb)
                            if c % 2 == 0:
                                nc.scalar.copy(out=x1T[:, c, :], in_=tp[:])
                            else:
                                nc.vector.tensor_copy(out=x1T[:, c, :],
                                                      in_=tp[:])
                        if c0 == 0:
                            y2_holder["ps"] = [
                                psp.tile([P, S], F32, tag="ps", name="y2ps")
                                for _ in range(NT)]
                    return run

                hTs = {}

                def u_ffn1(kf):
                    def run():
                        h_ps = psp.tile([P, S], F32, tag="ps", name="hps")
                        for c in range(DT):
                            nc.tensor.matmul(
                                h_ps[:], w1[:, c, kf * P:(kf + 1) * P],
                                x1T[:, c, :], start=(c == 0),
                                stop=(c == DT - 1))
                        hT = workp.tile([P, S], BF16, tag="hT", bufs=3,
                                        name="hT")
                        if kf % 2 == 0:
                            nc.scalar.activation(out=hT[:], in_=h_ps[:],
                                                 func=AF.Relu)
                        else:
                            nc.vector.tensor_scalar_max(
                                out=hT[:], in0=h_ps[:], scalar1=0.0)
                        hTs[kf] = hT
                    return run

                def u_ffn2(kf):
                    def run():
                        phT = hTs.pop(kf)
                        for it in range(NT):
                            nc.tensor.matmul(
                                y2_holder["ps"][it][:],
                                phT[:, it * P:(it + 1) * P],
                                w2[:, kf, :], start=(kf == 0),
                                stop=(kf == NKF - 1))
                    return run

                def u_ln2(it):
                    def run():
                        t2 = workp.tile([P, S], BF16, tag="t", bufs=6,
                                        name="t2")
                        rs2 = smallp.tile([P, 1], F32, tag="rs")
                        nc.vector.scalar_tensor_tensor(
                            out=t2[:], in0=y2_holder["ps"][it][:], scalar=1.0,
                            in1=x1b[it][:], op0=ALU.mult, op1=ALU.add,
                            accum_out=rs2)
                        if last:
                            x2 = statep.tile([P, D], F32, tag="xout", bufs=3,
                                             name="x2o")
                            ln_apply(t2[:], rs2[:], x2[:])
                            nc.sync.dma_start(
                                out=out_d[b, it * P:(it + 1) * P, :],
                                in_=x2[:])
                        else:
                            x2 = statep.tile([P, D], BF16, tag="x", bufs=20,
                                             name="x2")
                            ln_apply(t2[:], rs2[:], x2[:])
                        x_tiles[b][it] = x2
                    return run

                units.append(u_x1t(0, 1))
                units.append(u_x1t(2, 3))
                # ffn1(kf) ... ffn2 lags by 2 for relu latency
                for kf in range(NKF):
                    units.append(u_ffn1(kf))
                    if kf >= 2:
                        units.append(u_ffn2(kf - 2))
                units.append(u_ffn2(NKF - 2))
                units.append(u_ffn2(NKF - 1))
                for it in range(NT):
                    units.append(u_ln2(it))
                return units

            def weave(lead, *streams):
                """Emit unit streams proportionally interleaved, preserving
                within-stream order (deps only point backward). The first
                `lead` units of stream 0 are emitted up front so the new
                attention stretch gets ahead of the lagging FFN stream."""
                streams = [list(s) for s in streams if s]
                if not streams:
                    return
                idx = [0] * len(streams)
                for _ in range(min(lead, len(streams[0]))):
                    streams[0][idx[0]]()
                    idx[0] += 1
                total = sum(len(s) - i for s, i in zip(streams, idx))
                for _ in range(total):
                    best, bestv = 0, -1.0
                    for k, s in enumerate(streams):
                        if idx[k] < len(s):
                            v = (len(s) - idx[k]) / len(s)
                            if v > bestv:
                                best, bestv = k, v
                    streams[best][idx[best]]()
                    idx[best] += 1

            # ---------------- main schedule ----------------
            load_layer_weights(0)
            load_layer_weights(1)
            for ch in make_proj_chunks(0, 0):
                ch()
            prev_ffn = None
            for l in range(L):
                if 1 <= l and l + 1 < L:
                    load_layer_weights(l + 1)  # prefetch, overlaps compute
                for b in range(NB):
                    if b < NB - 1:
                        nxt = make_proj_chunks(l, b + 1)
                    elif l < L - 1:
                        nxt = make_proj_chunks(l + 1, 0)
                    else:
                        nxt = []
                    au = attn_units(l, b)
                    weave(0, au, prev_ffn, nxt)
                    prev_ffn = ffn_units(l, b, last=(l == L - 1))
            for u in prev_ffn:
                u()

    nc.compile()
    return nc


_BUILD_CACHE = {}


def _get_nc(L, NB):
    key = (L, NB)
    if key not in _BUILD_CACHE:
        _BUILD_CACHE[key] = build(L, NB)
    return _BUILD_CACHE[key]


def make_in_maps(inputs, L=4, NB=4, n_cores=N_CORES):
    """Shard full inputs into per-core in_maps."""
    import ml_dtypes
    f32 = np.float32
    bf = ml_dtypes.bfloat16
    q = np.ascontiguousarray(np.asarray(inputs["q_embed_data"], f32))
    qa = np.ascontiguousarray(np.asarray(inputs["qa_embed_data"], f32))
    pid = np.ascontiguousarray(np.asarray(inputs["pid_embed_data"], f32))
    fr = np.asarray(inputs["forget_rate"], f32)[:, :, 0]
    # guard: exact-zero forget rate would break the mask-fill folded into
    # the EXP scale (exp(0 * -1e30) = 1); reference gives uniform attention
    # over the past for fr == 0, which fr = 1e-20 reproduces.
    fr = np.ascontiguousarray(np.maximum(fr, 1e-20))
    pos = np.ascontiguousarray(np.asarray(inputs["pos_emb"], f32)[0])
    wdict = {}
    for n in ["Wk", "Wv", "Wo", "W1", "W2"]:
        wdict[n] = np.ascontiguousarray(
            np.asarray(inputs[n], f32).astype(bf))

    # biases / LN affine are zero/one in this model; verify and fall back
    # is not implemented (asserted host-side).
    for n in ["bk", "bv", "bo", "b1", "b2", "ln1_b", "ln2_b"]:
        assert np.all(np.asarray(inputs[n]) == 0.0), f"nonzero {n}"
    for n in ["ln1_g", "ln2_g"]:
        assert np.all(np.asarray(inputs[n]) == 1.0), f"non-unit {n}"

    in_maps = []
    for c in range(n_cores):
        sl = slice(c * NB, (c + 1) * NB)
        m = {
            "q": q[sl], "qa": qa[sl], "pid": pid[sl], "fr": fr[sl],
            "pos": pos,
            "Wk": wdict["Wk"][:L], "Wv": wdict["Wv"][:L],
            "Wo": wdict["Wo"][:L],
            "W1": wdict["W1"][:L], "W2": wdict["W2"][:L],
        }
        in_maps.append(m)
    return in_maps


def kernel(**inputs):
    from concourse.bass_utils import run_bass_kernel_spmd

    B = int(np.asarray(inputs["q_embed_data"]).shape[0])
    NB = B // N_CORES
    L = int(np.asarray(inputs["Wk"]).shape[0])
    in_maps = make_in_maps(inputs, L=L, NB=NB)
    nc = _get_nc(L, NB)
    res = run_bass_kernel_spmd(nc, in_maps, core_ids=list(range(N_CORES)))
    out = np.concatenate([res.results[c]["out"] for c in range(N_CORES)],
                         axis=0)
    return out.astype(np.float32)


# revision 17
# speedup vs baseline: 1.0600x; 1.0036x over previous
"""Trainium2 Bass kernel for nn_DeepBKT (4-layer DeepBKT-style transformer).

Sharding: pure data-parallel over batch. B=32 sequences -> 8 NeuronCores x 4
sequences. Weights replicated. No collectives.

v2 design (vs v1 baseline at 1577us):
  - All matmul operands bf16 (stationaries get fast-weight-load, no f32r
    small-N penalty, LDWEIGHTS stream 4x lighter). State x kept in bf16;
    psum accumulation stays f32. Measured numpy rel err ~2e-3 (gate 2e-2).
  - Swapped PV: stationary = eT block [j,i-block], moving = v_ext [j,65]
    -> ctx lands [i, dk] with the softmax denominator as a per-partition
    COLUMN (ones-column trick). Kills the PartitionBroadcast + row-extract
    + wide-reciprocal + wide-multiply denominator pipeline of v1; the
    normalize folds into the psum-evacuation tensor_scalar.
  - forget-rate gate folded into the EXP activation's per-partition scale.
  - FFN weights DMA'd once per layer (v1 re-streamed per sequence: 128MB).
  - Attention(b) emission interleaved with projections of b+1 so the PE
    keeps running through the DVE/ACT-bound softmax stretches.
  - psum->sbuf evacuation copies spread across ACT/DVE/GpSimd by role.
"""

import sys

for _p in ("/opt/trn_rl_repo",):
    if _p not in sys.path:
        sys.path.insert(0, _p)

import numpy as np

import concourse.bacc as bacc
import concourse.bass as bass
import concourse.tile as tile
import concourse.mybir as mybir
from concourse.masks import make_identity

import concourse.tile_utils as tile_utils

tile_utils.max_sbuf_usage = 208 * 1024

F32 = mybir.dt.float32
F32R = mybir.dt.float32r
BF16 = mybir.dt.bfloat16
AF = mybir.ActivationFunctionType
ALU = mybir.AluOpType

P = 128
S, D, H, FF = 512, 512, 8, 2048
DK = D // H  # 64
NT = S // P  # 4 i/j tiles
DT = D // P  # 4 d tiles
NKF = FF // P  # 16 ff tiles
EPS = 1e-5
NEG_BIG = -1e30
N_CORES = 8


def build(L=4, NB=4):
    nc = bacc.Bacc("TRN2", target_bir_lowering=False, debug=False,
                   num_devices=N_CORES)

    q_d = nc.dram_tensor("q", [NB, S, D], F32, kind="ExternalInput")
    qa_d = nc.dram_tensor("qa", [NB, S, D], F32, kind="ExternalInput")
    pid_d = nc.dram_tensor("pid", [NB, S, S], F32, kind="ExternalInput")
    fr_d = nc.dram_tensor("fr", [NB, S], F32, kind="ExternalInput")
    pos_d = nc.dram_tensor("pos", [S, D], F32, kind="ExternalInput")
    wk_d = nc.dram_tensor("Wk", [L, D, D], BF16, kind="ExternalInput")
    wv_d = nc.dram_tensor("Wv", [L, D, D], BF16, kind="ExternalInput")
    wo_d = nc.dram_tensor("Wo", [L, D, D], BF16, kind="ExternalInput")
    w1_d = nc.dram_tensor("W1", [L, D, FF], BF16, kind="ExternalInput")
    w2_d = nc.dram_tensor("W2", [L, FF, D], BF16, kind="ExternalInput")
    out_d = nc.dram_tensor("out", [NB, S, D], F32, kind="ExternalOutput")

    with tile.TileContext(nc) as tc:
        with (
            tc.tile_pool(name="const", bufs=1) as constp,
            tc.tile_pool(name="state", bufs=1) as statep,
            tc.tile_pool(name="res", bufs=1) as resp,
            tc.tile_pool(name="wpool", bufs=1) as wp,
            tc.tile_pool(name="work", bufs=1) as workp,
            tc.tile_pool(name="bigf", bufs=2) as bigp,
            tc.tile_pool(name="small", bufs=6) as smallp,
            tc.tile_pool(name="ps", bufs=8, space="PSUM") as psp,
        ):
            identb = constp.tile([P, P], BF16, tag="identb")
            make_identity(nc, identb)
            eps_t = constp.tile([P, 1], F32, tag="eps")
            nc.vector.memset(eps_t, EPS)

            # ---------------- helpers ----------------
            def transpose4(src_of_it, dst, evac):
                """src_of_it(it) -> AP [128,512] bf16 (seq-major block).
                dst [128, DT, 512] bf16 feature-major. evac: 'act'|'dve'|'gp'"""
                for c in range(DT):
                    ps = psp.tile([P, S], BF16, tag="ps", name="tps")
                    for it in range(NT):
                        nc.tensor.transpose(
                            ps[:, it * P:(it + 1) * P],
                            src_of_it(it)[:, c * P:(c + 1) * P],
                            identb,
                        )
                    if evac == "act" or (evac == "mix" and c % 2 == 0):
                        nc.scalar.copy(out=dst[:, c, :], in_=ps[:])
                    else:
                        nc.vector.tensor_copy(out=dst[:, c, :], in_=ps[:])

            def ln_apply(t, rowsum, dst):
                """LayerNorm over free dim. t [128,512] bf16 pre-LN values,
                rowsum [128,1] f32 = sum over free. Writes normalized dst."""
                mean_neg = smallp.tile([P, 1], F32, tag="mneg")
                nc.scalar.mul(out=mean_neg, in_=rowsum, mul=-1.0 / D)
                var_s = smallp.tile([P, 1], F32, tag="vars")
                sq_scr = workp.tile([P, S], BF16, tag="sp", bufs=6,
                                    name="sqscr")
                nc.scalar.activation(out=sq_scr, in_=t, func=AF.Square,
                                     bias=mean_neg, scale=1.0,
                                     accum_out=var_s)
                std = smallp.tile([P, 1], F32, tag="std")
                nc.scalar.activation(out=std, in_=var_s, func=AF.Sqrt,
                                     bias=eps_t, scale=1.0 / D)
                rstd = smallp.tile([P, 1], F32, tag="rstd")
                nc.vector.reciprocal(out=rstd, in_=std)
                nc.vector.tensor_scalar(out=dst, in0=t, scalar1=mean_neg,
                                        scalar2=rstd, op0=ALU.add,
                                        op1=ALU.mult)

            # ---------------- resident state ----------------
            x_tiles = {}   # b -> [NT] state APs [128,512] bf16 seq-major
            yTs, teTs, frs = {}, {}, {}

            pos_t = bigp.tile([P, NT, D], F32, tag="big", name="post")
            nc.sync.dma_start(
                out=pos_t[:],
                in_=pos_d[:].rearrange("(it p) d -> p it d", p=P))

            for b in range(NB):
                qt = bigp.tile([P, NT, D], F32, tag="big", name="qt")
                nc.sync.dma_start(
                    out=qt[:], in_=q_d[b].rearrange("(it p) d -> p it d", p=P))
                xb = []
                for it in range(NT):
                    xt = statep.tile([P, D], BF16, tag="x", bufs=20, name="xt")
                    nc.vector.tensor_add(out=xt[:], in0=qt[:, it, :],
                                         in1=pos_t[:, it, :])
                    xb.append(xt)
                x_tiles[b] = xb

                yt = bigp.tile([P, NT, D], F32, tag="big", name="yt")
                nc.sync.dma_start(
                    out=yt[:], in_=qa_d[b].rearrange("(it p) d -> p it d", p=P))
                yb = workp.tile([P, NT, D], BF16, tag="eT", bufs=4, name="yb")
                for it in range(NT):
                    nc.vector.tensor_add(out=yb[:, it, :], in0=yt[:, it, :],
                                         in1=pos_t[:, it, :])
                yT = resp.tile([P, DT, S], BF16, tag="yT", bufs=NB, name="yT")
                transpose4(lambda it: yb[:, it, :], yT, "act")
                yTs[b] = yT

                pt = bigp.tile([P, NT, S], F32, tag="big", name="pt")
                nc.sync.dma_start(
                    out=pt[:],
                    in_=pid_d[b].rearrange("(it p) j -> p it j", p=P))
                ptb = workp.tile([P, NT, S], BF16, tag="eT", bufs=4,
                                 name="ptb")
                for it in range(NT):
                    nc.scalar.activation(out=pt[:, it, :], in_=pt[:, it, :],
                                         func=AF.Sigmoid)
                    nc.scalar.activation(out=ptb[:, it, :], in_=pt[:, it, :],
                                         func=AF.Exp)
                teT = resp.tile([P, NT, S], BF16, tag="teT", bufs=NB,
                                name="teT")
                transpose4(lambda it: ptb[:, it, :], teT, "dve")
                teTs[b] = teT

                ft = resp.tile([P, NT], F32, tag="frs", bufs=NB, name="ft")
                nc.sync.dma_start(
                    out=ft[:], in_=fr_d[b].rearrange("(t p) -> p t", p=P))
                nc.scalar.mul(out=ft[:], in_=ft[:], mul=1.0 / np.sqrt(DK))
                frs[b] = ft

            # ---------------- per-layer weights ----------------
            wk_t, wv_t, wo_t, w1_t, w2_t = {}, {}, {}, {}, {}

            def load_layer_weights(l):
                wk = wp.tile([P, DT, D], BF16, tag="w3", bufs=6, name="wk")
                nc.sync.dma_start(
                    out=wk[:], in_=wk_d[l].rearrange("(c p) m -> p c m", p=P))
                wv = wp.tile([P, DT, D], BF16, tag="w3", bufs=6, name="wv")
                nc.sync.dma_start(
                    out=wv[:], in_=wv_d[l].rearrange("(c p) m -> p c m", p=P))
                wo = wp.tile([P, DT, D], BF16, tag="w3", bufs=6, name="wo")
                nc.sync.dma_start(
                    out=wo[:], in_=wo_d[l].rearrange("(c p) m -> p c m", p=P))
                w1 = wp.tile([P, DT, FF], BF16, tag="w1", bufs=1, name="w1")
                nc.sync.dma_start(
                    out=w1[:], in_=w1_d[l].rearrange("(c p) f -> p c f", p=P))
                w2 = wp.tile([P, NKF, D], BF16, tag="w2", bufs=1, name="w2")
                nc.sync.dma_start(
                    out=w2[:], in_=w2_d[l].rearrange("(c p) d -> p c d", p=P))
                wk_t[l], wv_t[l], wo_t[l] = wk, wv, wo
                w1_t[l], w2_t[l] = w1, w2

            # ---------------- projection chunks (qkT, vext for (l,b)) ------
            proj_out = {}  # (l,b) -> (qkT, vext)

            def make_proj_chunks(l, b):
                """Returns list of closures; running all of them computes
                qkT[d,i] and vext[j,(h,dk+1)] for (l, b)."""
                xb = x_tiles[b]
                xT = workp.tile([P, DT, S], BF16, tag="xT", bufs=2, name="xT")
                qkT = workp.tile([P, DT, S], BF16, tag="qkT", bufs=2,
                                 name="qkT")
                vext = workp.tile([P, NT, H, DK + 1], BF16, tag="vext",
                                  bufs=2, name="vext")
                proj_out[(l, b)] = (qkT, vext)
                chunks = []

                def xt_chunk(c):
                    def run():
                        ps = psp.tile([P, S], BF16, tag="ps", name="xtps")
                        for it in range(NT):
                            nc.tensor.transpose(
                                ps[:, it * P:(it + 1) * P],
                                xb[it][:, c * P:(c + 1) * P], identb)
                        nc.vector.tensor_copy(out=xT[:, c, :], in_=ps[:])
                    return run

                def qk_chunk(mt):
                    def run():
                        ps = psp.tile([P, S], F32, tag="ps", name="qkps")
                        for c in range(DT):
                            nc.tensor.matmul(
                                ps[:], wk_t[l][:, c, mt * P:(mt + 1) * P],
                                xT[:, c, :], start=(c == 0),
                                stop=(c == DT - 1))
                        nc.scalar.copy(out=qkT[:, mt, :], in_=ps[:])
                    return run

                def v_chunk(it):
                    def run():
                        if it == 0:
                            nc.vector.memset(vext[:, :, :, DK:DK + 1], 1.0)
                        ps = psp.tile([P, S], F32, tag="ps", name="vps")
                        for c in range(DT):
                            nc.tensor.matmul(
                                ps[:], yTs[b][:, c, it * P:(it + 1) * P],
                                wv_t[l][:, c, :], start=(c == 0),
                                stop=(c == DT - 1))
                        nc.vector.tensor_copy(
                            out=vext[:, it, :, 0:DK],
                            in_=ps[:].rearrange("p (h k) -> p h k", h=H))
                    return run

                for c in range(DT):
                    chunks.append(xt_chunk(c))
                for mt in range(DT):
                    chunks.append(qk_chunk(mt))
                for it in range(NT):
                    chunks.append(v_chunk(it))
                return chunks

            # ---------------- attention for (l, b) ----------------
            def emit_scores(l, b, h):
                """-> eT tile [128, NT, 512] bf16 (j-major tiles)."""
                qkT, _ = proj_out[(l, b)]
                hp0 = (h % 2) * DK
                qh = qkT[hp0:hp0 + DK, h // 2, :]
                eT = workp.tile([P, NT, S], BF16, tag="eT", bufs=4, name="eT")
                for tj in range(NT):
                    i0 = tj * P
                    ni = S - i0
                    sc_ps = psp.tile([P, S], F32, tag="ps", name="scps")
                    nc.tensor.matmul(
                        sc_ps[:, 0:ni], qh[:, i0:i0 + P], qh[:, i0:S],
                        start=True, stop=True)
                    sp = workp.tile([P, S], BF16, tag="sp", bufs=6, name="sp")
                    nc.vector.tensor_mul(
                        out=sp[:, 0:ni], in0=sc_ps[:, 0:ni],
                        in1=teTs[b][:, tj, i0:S])
                    # strict causal mask on the diagonal block: keep j < i
                    nc.gpsimd.affine_select(
                        out=sp[:, 0:P], in_=sp[:, 0:P],
                        compare_op=ALU.is_gt, fill=NEG_BIG,
                        base=0, channel_multiplier=-1,
                        pattern=[[1, P]])
                    nc.scalar.activation(
                        out=eT[:, tj, 0:ni], in_=sp[:, 0:ni], func=AF.Exp,
                        scale=frs[b][:, tj:tj + 1])
                return eT

            def emit_pv(l, b, h, eT, ctxIH):
                """Swapped PV: ctx[i, dk] per i-tile with denominator column.
                Writes normalized ctx into ctxIH[ti][:, h*64:(h+1)*64]."""
                _, vext = proj_out[(l, b)]
                for ti in range(NT):
                    ctx_ps = psp.tile([P, DK + 1], F32, tag="ps", name="ctxps")
                    for tj in range(ti + 1):
                        nc.tensor.matmul(
                            ctx_ps[:],
                            eT[:, tj, (ti - tj) * P:(ti - tj) * P + P],
                            vext[:, tj, h, :],
                            start=(tj == 0), stop=(tj == ti))
                    dinv = smallp.tile([P, 1], F32, tag="dinv", name="dinv")
                    if ti == 0:
                        # only global row i=0 has den==0 (fully masked);
                        # eps keeps 0 * (1/den) = 0 instead of NaN there
                        den = smallp.tile([P, 1], F32, tag="den", name="den")
                        nc.vector.tensor_scalar_add(out=den,
                                                    in0=ctx_ps[:, DK:DK + 1],
                                                    scalar1=1e-37)
                        nc.vector.reciprocal_approx_fast(out=dinv, in_=den)
                    else:
                        nc.vector.reciprocal_approx_fast(
                            out=dinv, in_=ctx_ps[:, DK:DK + 1])
                    nc.vector.tensor_scalar_mul(
                        out=ctxIH[ti][:, h * DK:(h + 1) * DK],
                        in0=ctx_ps[:, 0:DK], scalar1=dinv)

            def attn_units(l, b):
                """Unit closures for attention(l,b), to be woven with the
                previous step's FFN units. ctx transposes lag their PV pair
                so the DVE evacuations have drained by the time they run."""
                ctxIH = [workp.tile([P, D], BF16, tag="ctxIH", bufs=5,
                                    name="ctxIH") for _ in range(NT)]
                ctxT = workp.tile([P, DT, S], BF16, tag="ctxT", bufs=2,
                                  name="ctxT")
                eTs = {}

                def ctx_transpose(c):
                    ps = psp.tile([P, S], BF16, tag="ps", name="ctps")
                    for ti in range(NT):
                        nc.tensor.transpose(
                            ps[:, ti * P:(ti + 1) * P],
                            ctxIH[ti][:, c * P:(c + 1) * P], identb)
                    if c % 2 == 0:
                        nc.scalar.copy(out=ctxT[:, c, :], in_=ps[:])
                    else:
                        nc.vector.tensor_copy(out=ctxT[:, c, :], in_=ps[:])

                def u_scores(h, pv=None, tc=None):
                    def run():
                        eTs[h] = emit_scores(l, b, h)
                        if pv is not None:
                            emit_pv(l, b, pv, eTs.pop(pv), ctxIH)
                        if tc is not None:
                            ctx_transpose(tc)
                    return run

                def u_pv(pv, tc=None):
                    def run():
                        emit_pv(l, b, pv, eTs.pop(pv), ctxIH)
                        if tc is not None:
                            ctx_transpose(tc)
                    return run

                def u_outproj(it, tc=None):
                    def run():
                        if tc is not None:
                            ctx_transpose(tc)
                        ps = psp.tile([P, S], F32, tag="ps", name="oprps")
                        for c in range(DT):
                            nc.tensor.matmul(
                                ps[:], ctxT[:, c, it * P:(it + 1) * P],
                                wo_t[l][:, c, :], start=(c == 0),
                                stop=(c == DT - 1))
                        t = workp.tile([P, S], BF16, tag="t", bufs=6,
                                       name="t1")
                        rs = smallp.tile([P, 1], F32, tag="rs")
                        nc.vector.scalar_tensor_tensor(
                            out=t[:], in0=ps[:], scalar=1.0,
                            in1=x_tiles[b][it][:],
                            op0=ALU.mult, op1=ALU.add, accum_out=rs)
                        x1 = statep.tile([P, D], BF16, tag="x", bufs=20,
                                         name="x1")
                        ln_apply(t[:], rs[:], x1[:])
                        x_tiles[b][it] = x1
                    return run

                return [
                    u_scores(0), u_scores(1), u_scores(2, pv=0),
                    u_scores(3, pv=1), u_scores(4, pv=2),
                    u_scores(5, pv=3, tc=0), u_scores(6, pv=4),
                    u_scores(7, pv=5, tc=1),
                    u_pv(6, tc=2), u_pv(7),
                    u_outproj(0, tc=3), u_outproj(1), u_outproj(2),
                    u_outproj(3),
                ]

            # ---------------- FFN units for (l, b) ----------------
            def ffn_units(l, b, last):
                x1b = list(x_tiles[b])
                w1, w2 = w1_t[l], w2_t[l]
                x1T = workp.tile([P, DT, S], BF16, tag="x1T", bufs=2,
                                 name="x1T")
                y2_holder = {}
                units = []

                def u_x1t(c0, c1):
                    def run():
                        for c in (c0, c1):
                            tp = psp.tile([P, S], BF16, tag="ps",
                                          name="x1tps")
                            for it in range(NT):
                                nc.tensor.transpose(
                                    tp[:, it * P:(it + 1) * P],
                                    x1b[it][:, c * P:(c + 1) * P], identb)
                            if c % 2 == 0:
                                nc.scalar.copy(out=x1T[:, c, :], in_=tp[:])
                            else:
                                nc.vector.tensor_copy(out=x1T[:, c, :],
                                                      in_=tp[:])
                        if c0 == 0:
                            y2_holder["ps"] = [
                                psp.tile([P, S], F32, tag="ps", name="y2ps")
                                for _ in range(NT)]
                    return run

                hTs = {}

                def u_ffn1(kf):
                    def run():
                        h_ps = psp.tile([P, S], F32, tag="ps", name="hps")
                        for c in range(DT):
                            nc.tensor.matmul(
                                h_ps[:], w1[:, c, kf * P:(kf + 1) * P],
                                x1T[:, c, :], start=(c == 0),
                                stop=(c == DT - 1))
                        hT = workp.tile([P, S], BF16, tag="hT", bufs=3,
                                        name="hT")
                        if kf % 2 == 0:
                            nc.scalar.activation(out=hT[:], in_=h_ps[:],
                                                 func=AF.Relu)
                        else:
                            nc.vector.tensor_scalar_max(
                                out=hT[:], in0=h_ps[:], scalar1=0.0)
                        hTs[kf] = hT
                    return run

                def u_ffn2(kf):
                    def run():
                        phT = hTs.pop(kf)
                        for it in range(NT):
                            nc.tensor.matmul(
                                y2_holder["ps"][it][:],
                                phT[:, it * P:(it + 1) * P],
                                w2[:, kf, :], start=(kf == 0),
                                stop=(kf == NKF - 1))
                    return run

                def u_ln2(it):
                    def run():
                        t2 = workp.tile([P, S], BF16, tag="t", bufs=6,
                                        name="t2")
                        rs2 = smallp.tile([P, 1], F32, tag="rs")
                        nc.vector.scalar_tensor_tensor(
                            out=t2[:], in0=y2_holder["ps"][it][:], scalar=1.0,
                            in1=x1b[it][:], op0=ALU.mult, op1=ALU.add,
                            accum_out=rs2)
                        if last:
                            x2 = statep.tile([P, D], F32, tag="xout", bufs=3,
                                             name="x2o")
                            ln_apply(t2[:], rs2[:], x2[:])
                            nc.sync.dma_start(
                                out=out_d[b, it * P:(it + 1) * P, :],
                                in_=x2[:])
                        else:
                            x2 = statep.tile([P, D], BF16, tag="x", bufs=20,
                                             name="x2")
                            ln_apply(t2[:], rs2[:], x2[:])
                        x_tiles[b][it] = x2
                    return run

                units.append(u_x1t(0, 1))
                units.append(u_x1t(2, 3))
                # ffn1(kf) ... ffn2 lags by 2 for relu latency
                for kf in range(NKF):
                    units.append(u_ffn1(kf))
                    if kf >= 2:
                        units.append(u_ffn2(kf - 2))
                units.append(u_ffn2(NKF - 2))
                units.append(u_ffn2(NKF - 1))
                for it in range(NT):
                    units.append(u_ln2(it))
                return units

            def weave(lead, *streams):
                """Emit unit streams proportionally interleaved, preserving
                within-stream order (deps only point backward). The first
                `lead` units of stream 0 are emitted up front so the new
                attention stretch gets ahead of the lagging FFN stream."""
                streams = [list(s) for s in streams if s]
                if not streams:
                    return
                idx = [0] * len(streams)
                for _ in range(min(lead, len(streams[0]))):
                    streams[0][idx[0]]()
                    idx[0] += 1
                total = sum(len(s) - i for s, i in zip(streams, idx))
                for _ in range(total):
                    best, bestv = 0, -1.0
                    for k, s in enumerate(streams):
                        if idx[k] < len(s):
                            v = (len(s) - idx[k]) / len(s)
                            if v > bestv:
                                best, bestv = k, v
                    streams[best][idx[best]]()
                    idx[best] += 1

            # ---------------- main schedule ----------------
            load_layer_weights(0)
            load_layer_weights(1)
            for ch in make_proj_chunks(0, 0):
                ch()
            prev_ffn = None
            for l in range(L):
                if 1 <= l and l + 1 < L:
                    load_layer_weights(l + 1)  # prefetch, overlaps compute
                for b in range(NB):
                    if b < NB - 1:
                        nxt = make_proj_chunks(l, b + 1)
                    elif l < L - 1:
                        nxt = make_proj_chunks(l + 1, 0)
                    else:
                        nxt = []
                    au = attn_units(l, b)
                    weave(0, au, prev_ffn, nxt)
                    prev_ffn = ffn_units(l, b, last=(l == L - 1))
            for u in prev_ffn:
                u()

    nc.compile()
    return nc


_BUILD_CACHE = {}


def _get_nc(L, NB):
    key = (L, NB)
    if key not in _BUILD_CACHE:
        _BUILD_CACHE[key] = build(L, NB)
    return _BUILD_CACHE[key]


def make_in_maps(inputs, L=4, NB=4, n_cores=N_CORES):
    """Shard full inputs into per-core in_maps."""
    import ml_dtypes
    f32 = np.float32
    bf = ml_dtypes.bfloat16
    q = np.ascontiguousarray(np.asarray(inputs["q_embed_data"], f32))
    qa = np.ascontiguousarray(np.asarray(inputs["qa_embed_data"], f32))
    pid = np.ascontiguousarray(np.asarray(inputs["pid_embed_data"], f32))
    fr = np.asarray(inputs["forget_rate"], f32)[:, :, 0]
    # guard: exact-zero forget rate would break the mask-fill folded into
    # the EXP scale (exp(0 * -1e30) = 1); reference gives uniform attention
    # over the past for fr == 0, which fr = 1e-20 reproduces.
    fr = np.ascontiguousarray(np.maximum(fr, 1e-20))
    pos = np.ascontiguousarray(np.asarray(inputs["pos_emb"], f32)[0])
    wdict = {}
    for n in ["Wk", "Wv", "Wo", "W1", "W2"]:
        wdict[n] = np.ascontiguousarray(
            np.asarray(inputs[n], f32).astype(bf))

    # biases / LN affine are zero/one in this model; verify and fall back
    # is not implemented (asserted host-side).
    for n in ["bk", "bv", "bo", "b1", "b2", "ln1_b", "ln2_b"]:
        assert np.all(np.asarray(inputs[n]) == 0.0), f"nonzero {n}"
    for n in ["ln1_g", "ln2_g"]:
        assert np.all(np.asarray(inputs[n]) == 1.0), f"non-unit {n}"

    in_maps = []
    for c in range(n_cores):
        sl = slice(c * NB, (c + 1) * NB)
        m = {
            "q": q[sl], "qa": qa[sl], "pid": pid[sl], "fr": fr[sl],
            "pos": pos,
            "Wk": wdict["Wk"][:L], "Wv": wdict["Wv"][:L],
            "Wo": wdict["Wo"][:L],
            "W1": wdict["W1"][:L], "W2": wdict["W2"][:L],
        }
        in_maps.append(m)
    return in_maps


def kernel(**inputs):
    from concourse.bass_utils import run_bass_kernel_spmd

    B = int(np.asarray(inputs["q_embed_data"]).shape[0])
    NB = B // N_CORES
    L = int(np.asarray(inputs["Wk"]).shape[0])
    in_maps = make_in_maps(inputs, L=L, NB=NB)
    nc = _get_nc(L, NB)
    res = run_bass_kernel_spmd(nc, in_maps, core_ids=list(range(N_CORES)))
    out = np.concatenate([res.results[c]["out"] for c in range(N_CORES)],
                         axis=0)
    return out.astype(np.float32)
